# revision 17
# baseline (speedup 1.0000x reference)
"""GNN NodeBlock (segment_sum scatter + 2-layer MLP) on 8 Trainium2 cores.

v2 strategy (edge/vertex partitioning by receiver range, fp8 payload):
 - 2 graphs x 4 cores each; core owns a 12500-node range and all edges
   whose receiver falls in that range.
 - Nodes grouped in 128-node blocks (98/core), each split into 4 stripes
   of 32 nodes. Host buckets edges by stripe; each stripe owns 3 fixed
   128-edge chunks (12 chunks/block). Stripes overflowing 384 edges are
   pre-compressed host-side (tail summed by receiver, <=32 rows).
 - Edge features are quantized to fp8 e4m3 with host-side error
   feedback: the fp8 rounding residual of each edge is carried into the
   next edge targeting the same (node, feature), so the device's exact
   fp32 PSUM sum of the quantized edges matches the fp32 sum to ~1 ulp
   of a single fp8 value. Halves DMA bytes vs bf16 at ~6e-3 rel error.
 - Edge ids are stored as fp8 BIT PATTERNS 1..32 (distinct values, not
   integers) and compared against a host-provided fp8 pattern iota via
   DVE is_equal to build the per-chunk one-hot [128 edge, 32 node].
 - Device, per chunk: scatter via matmul into agg_T [De, 128]
   (feature-major, fp8 lhsT/rhs, fp32 PSUM), then the MLP feature-major
   in bf16, batched over groups of 4 blocks.
 - Engine split: one-hot + agg PSUM->SBUF copy on Vector, relu+bias and
   out bias on Scalar, payload DMA on Sync queue, node/out DMA on
   GpSimd queue.
"""
import numpy as np
import ml_dtypes

import concourse.bacc as bacc
import concourse.mybir as mybir
from concourse.tile import TileContext
from concourse.bass_utils import run_bass_kernel_spmd

B, N, E = 2, 50000, 512000
De, Dv, H, Do = 128, 128, 256, 128
NCORES = 8
CPG = 4                    # cores per graph
NPC = N // CPG             # 12500 nodes per core
NB = (NPC + 127) // 128    # 98 blocks per core
SW = 32                    # stripe width: nodes per one-hot stripe
NSPB = 4                   # stripes per block
NSC = 11                   # 128-edge chunks per block: stripe blk%4 gets 2,
                           # the other three stripes get 3 (rotating short
                           # stripe; its tail is folded host-side)
IDOFF = NSC * 128          # ids start at column 1408
PAYW = IDOFF + NSC + 5     # 1424 bytes per partition row (5 pad)
GRP = 4                    # blocks per MLP group
GROUPS = [GRP] * (NB // GRP) + ([NB % GRP] if NB % GRP else [])

F32 = mybir.dt.float32
BF16 = mybir.dt.bfloat16
FP8 = mybir.dt.float8e4
NP_FP8 = ml_dtypes.float8_e4m3fn
NP_BF16 = ml_dtypes.bfloat16


def _build_nc():
    nc = bacc.Bacc("TRN2", target_bir_lowering=False)
    payload = nc.dram_tensor("payload", [len(GROUPS), 128, GRP * PAYW], FP8, kind="ExternalInput")
    nodes_g = nc.dram_tensor("nodes_g", [len(GROUPS), 128, GRP * 128], FP8, kind="ExternalInput")
    w1e = nc.dram_tensor("w1e", [128, 256], BF16, kind="ExternalInput")  # W1[:128] (agg half)
    w1n = nc.dram_tensor("w1n", [128, 256], FP8, kind="ExternalInput")   # W1[128:] (node half)
    w2 = nc.dram_tensor("w2", [128, 256], BF16, kind="ExternalInput")   # [p, hm*128+j] = W2[hm*128+p, j]
    b1 = nc.dram_tensor("b1", [128, 2], F32, kind="ExternalInput")      # [p, hm] = b1[hm*128+p]
    b2 = nc.dram_tensor("b2", [128, 1], F32, kind="ExternalInput")
    iota8 = nc.dram_tensor("iota8", [128, SW], FP8, kind="ExternalInput")
    out_g = nc.dram_tensor("out_g", [len(GROUPS), 128, GRP * 128], BF16, kind="ExternalOutput")

    with TileContext(nc) as tc:
        with tc.tile_pool(name="const", bufs=1) as cp, \
             tc.tile_pool(name="pay", bufs=12) as payp, \
             tc.tile_pool(name="oh", bufs=4) as ohp, \
             tc.tile_pool(name="nod", bufs=8) as nodp, \
             tc.tile_pool(name="agg4", bufs=3) as aggp, \
             tc.tile_pool(name="hsb", bufs=3) as hp, \
             tc.tile_pool(name="osb", bufs=4) as op_, \
             tc.tile_pool(name="psA", bufs=2, space="PSUM") as psA, \
             tc.tile_pool(name="psH", bufs=2, space="PSUM") as psH, \
             tc.tile_pool(name="psO", bufs=2, space="PSUM") as psO:
            # iota first (first one-hot depends on it), then weights on the
            # gpsimd queue so the sync queue starts streaming payload groups
            # immediately.
            iota_sb = cp.tile([128, SW], FP8)
            nc.sync.dma_start(out=iota_sb[:], in_=iota8[:, :])
            w1e_sb = cp.tile([128, 256], BF16)
            nc.gpsimd.dma_start(out=w1e_sb[:], in_=w1e[:, :])
            w1n_sb = cp.tile([128, 256], FP8)
            nc.gpsimd.dma_start(out=w1n_sb[:], in_=w1n[:, :])
            w2_sb = cp.tile([128, 256], BF16)
            nc.gpsimd.dma_start(out=w2_sb[:], in_=w2[:, :])
            b1_sb = cp.tile([128, 2], F32)
            nc.gpsimd.dma_start(out=b1_sb[:], in_=b1[:, :])
            b2_sb = cp.tile([128, 1], F32)
            nc.gpsimd.dma_start(out=b2_sb[:], in_=b2[:, :])

            # 3-stage software pipeline over groups. Per emission step t:
            #   load(t): DMAs + one-hot     (Vector does nothing else before
            #                                the next TT, so it runs at DMA
            #                                pace, never behind compute)
            #   scatter(t): 48 fp8 matmuls  (PE)
            #   mlp_h(t-1): CAST + h-MMs + relu  (deps finished during
            #                                     scatter(t-1)/this step)
            #   mlp_o(t-2): out-MMs
            #   fin(t-3): out bias + DMA
            # This keeps every engine FIFO free of waits on work emitted
            # later in its own stream, and the PE queue free of matmuls
            # whose inputs aren't already in flight on another engine.
            st = {}

            def load(gi, g_sz):
                s = st[gi] = {}
                nod = s["nod"] = nodp.tile([128, GRP * 128], FP8, name="nod")
                nc.gpsimd.dma_start(out=nod[:, :g_sz * 128],
                                    in_=nodes_g[gi, :, :g_sz * 128])
                pay_g = s["pay"] = payp.tile([128, GRP * PAYW], FP8, name="pay_g")
                nc.sync.dma_start(out=pay_g[:, :g_sz * PAYW],
                                  in_=payload[gi, :, :g_sz * PAYW])
                # one-hot for the whole group in one DVE op:
                # oh_g[p, g, c, n] = (ids[p, g, c] == iota[p, n])
                oh_g = s["oh"] = ohp.tile([128, GRP * NSC * SW], FP8, name="oh_g")
                ids_ap = (pay_g[:, :g_sz * PAYW]
                          .rearrange("p (g w) -> p g w", g=g_sz)
                          [:, :, IDOFF:IDOFF + NSC]
                          .to_broadcast([128, g_sz, NSC, SW]))
                iota_ap = (iota_sb[:, None, :SW]
                           .to_broadcast([128, g_sz * NSC, SW])
                           .rearrange("p (g c) n -> p g c n", g=g_sz))
                nc.vector.tensor_tensor(
                    out=oh_g[:, :g_sz * NSC * SW].rearrange(
                        "p (g c n) -> p g c n", g=g_sz, c=NSC),
                    in0=ids_ap,
                    in1=iota_ap,
                    op=mybir.AluOpType.is_equal,
                )

            def scatter(gi, g_sz):
                s = st[gi]
                pay_g, oh_g = s["pay"], s["oh"]
                agg_ps = s["agg_ps"] = psA.tile([128, GRP * 128], F32, space="PSUM", name="agg_ps")
                for g in range(g_sz):
                    po = g * PAYW
                    oo = g * NSC * SW
                    sh = (gi * GRP + g) % NSPB   # short stripe of this block
                    pc = 0
                    for sp in range(NSPB):
                        col = g * 128 + sp * SW
                        ncs = 2 if sp == sh else 3
                        for k in range(ncs):
                            nc.tensor.matmul(
                                out=agg_ps[:, col:col + SW],
                                lhsT=pay_g[:, po + pc * 128:po + (pc + 1) * 128],
                                rhs=oh_g[:, oo + pc * SW:oo + (pc + 1) * SW],
                                start=(k == 0),
                                stop=(k == ncs - 1),
                            )
                            pc += 1

            def mlp_h(gi, g_sz):
                s = st[gi]
                agg_sb = aggp.tile([128, GRP * 128], BF16)
                nc.scalar.copy(agg_sb[:, :g_sz * 128], s["agg_ps"][:, :g_sz * 128])
                hps = psH.tile([128, 2 * GRP * 128], F32, space="PSUM")
                for hm in range(2):
                    nc.tensor.matmul(
                        out=hps[:, hm * GRP * 128:hm * GRP * 128 + g_sz * 128],
                        lhsT=w1e_sb[:, hm * 128:(hm + 1) * 128],
                        rhs=agg_sb[:, :g_sz * 128],
                        start=True, stop=False,
                    )
                    nc.tensor.matmul(
                        out=hps[:, hm * GRP * 128:hm * GRP * 128 + g_sz * 128],
                        lhsT=w1n_sb[:, hm * 128:(hm + 1) * 128],
                        rhs=s["nod"][:, :g_sz * 128],
                        start=False, stop=True,
                    )
                h_sb = s["h"] = hp.tile([128, 2 * GRP * 128], BF16, name="h_sb")
                for hm in range(2):
                    nc.scalar.activation(
                        out=h_sb[:, hm * GRP * 128:hm * GRP * 128 + g_sz * 128],
                        in_=hps[:, hm * GRP * 128:hm * GRP * 128 + g_sz * 128],
                        func=mybir.ActivationFunctionType.Relu,
                        bias=b1_sb[:, hm:hm + 1],
                    )

            def mlp_o(gi, g_sz):
                s = st[gi]
                ops = s["ops"] = psO.tile([128, GRP * 128], F32, space="PSUM", name="ops")
                for hm in range(2):
                    nc.tensor.matmul(
                        out=ops[:, :g_sz * 128],
                        lhsT=w2_sb[:, hm * 128:(hm + 1) * 128],
                        rhs=s["h"][:, hm * GRP * 128:hm * GRP * 128 + g_sz * 128],
                        start=(hm == 0),
                        stop=(hm == 1),
                    )

            def fin(gi, g_sz):
                s = st[gi]
                o_sb = op_.tile([128, GRP * 128], BF16)
                if gi % 2 == 0:
                    nc.scalar.activation(
                        out=o_sb[:, :g_sz * 128],
                        in_=s["ops"][:, :g_sz * 128],
                        func=mybir.ActivationFunctionType.Identity,
                        bias=b2_sb[:, 0:1],
                    )
                else:
                    nc.vector.tensor_scalar(
                        out=o_sb[:, :g_sz * 128],
                        in0=s["ops"][:, :g_sz * 128],
                        scalar1=b2_sb[:, 0:1],
                        scalar2=None,
                        op0=mybir.AluOpType.add,
                    )
                nc.gpsimd.dma_start(out=out_g[gi, :, :g_sz * 128],
                                    in_=o_sb[:, :g_sz * 128])
                del st[gi]

            n = len(GROUPS)
            for t in range(n + 3):
                if t < n:
                    load(t, GROUPS[t])
                if t - 3 >= 0:
                    fin(t - 3, GROUPS[t - 3])
                if t < n:
                    scatter(t, GROUPS[t])
                if 0 <= t - 1 < n:
                    mlp_h(t - 1, GROUPS[t - 1])
                if 0 <= t - 2 < n:
                    mlp_o(t - 2, GROUPS[t - 2])
    nc.compile()
    return nc


def _quantize_feedback(efeat, local):
    """fp8-quantize edge rows with per-(node,feature) error feedback.

    Rows sharing a receiver node are quantized sequentially, carrying the
    rounding residual into the next row, so the fp32 sum of the quantized
    rows tracks the fp32 sum of the originals to ~one fp8 rounding error.
    """
    order = np.argsort(local, kind="stable")
    ls = local[order]
    counts = np.bincount(ls, minlength=NPC)
    offs = np.zeros(NPC, np.int64)
    np.cumsum(counts[:-1], out=offs[1:])
    rank = np.arange(len(ls)) - offs[ls]
    q = np.empty((len(ls), De), NP_FP8)
    carry = np.zeros((NPC, De), np.float32)
    es = efeat[order]
    maxr = int(rank.max()) + 1 if len(rank) else 0
    for r in range(maxr):
        m = rank == r
        idx = ls[m]
        v = es[m] + carry[idx]
        qq = v.astype(NP_FP8)
        carry[idx] = v - qq.astype(np.float32)
        q[m] = qq
    out = np.empty_like(q)
    out[order] = q
    return out


def _prep_core(efeat, local, nodes_g_core):
    """Build one core's payload from its edges (efeat fp32, local in [0,NPC))."""
    blk = local >> 7
    w128 = local & 127
    stripe = blk * NSPB + (w128 >> 5)   # global stripe id, [0, NB*4)
    w32 = w128 & 31
    nstripes = NB * NSPB
    sidx = np.arange(nstripes)
    s_of = sidx % NSPB
    b_of = sidx // NSPB
    nch_s = np.where((b_of % NSPB) == s_of, 2, 3)   # chunks per stripe
    cap = nch_s * 128
    keepcap = cap - SW

    counts = np.bincount(stripe, minlength=nstripes)
    order = np.argsort(stripe, kind="stable")
    str_s = stripe[order]
    w32_s = w32[order]
    ef_s = efeat[order]
    offs = np.zeros(nstripes, np.int64)
    np.cumsum(counts[:-1], out=offs[1:])
    pos = np.arange(len(str_s)) - offs[str_s]

    # fold the tail of any stripe exceeding its capacity: the edges beyond
    # keepcap are summed by receiver (<=32 rows) so the stripe fits
    foldm = (counts > cap)[str_s] & (pos >= keepcap[str_s])
    if foldm.any():
        key = str_s[foldm] * SW + w32_s[foldm]
        seg = np.zeros((nstripes * SW, De), np.float32)
        np.add.at(seg, key, ef_s[foldm])
        pres = np.zeros(nstripes * SW, bool)
        pres[key] = True
        nk = np.nonzero(pres)[0]
        ef_s = np.concatenate([ef_s[~foldm], seg[nk]])
        str_s = np.concatenate([str_s[~foldm], (nk // SW).astype(str_s.dtype)])
        w32_s = np.concatenate([w32_s[~foldm], (nk % SW).astype(w32_s.dtype)])
        order2 = np.argsort(str_s, kind="stable")
        str_s = str_s[order2]
        w32_s = w32_s[order2]
        ef_s = ef_s[order2]
        counts = np.bincount(str_s, minlength=nstripes)
        offs = np.zeros(nstripes, np.int64)
        np.cumsum(counts[:-1], out=offs[1:])
        pos = np.arange(len(str_s)) - offs[str_s]

    local_n = (str_s // NSPB) * 128 + (str_s % NSPB) * SW + w32_s
    qfeat = _quantize_feedback(ef_s, local_n)

    chunk_base = (np.cumsum(nch_s) - nch_s) - NSC * b_of  # chunk base within block
    pc = chunk_base[str_s] + pos // 128      # physical chunk 0..10
    prow = pos % 128
    blk_s = str_s // NSPB

    payload = np.zeros((NB, 128, PAYW), np.uint8)
    feat_view = payload[:, :, :IDOFF].reshape(NB, 128, NSC, 128).view(NP_FP8)
    feat_view[blk_s, prow, pc, :] = qfeat
    id_view = payload[:, :, IDOFF:IDOFF + NSC]
    id_view[blk_s, prow, pc] = (w32_s + 1).astype(np.uint8)  # fp8 patterns
    ng = len(GROUPS)
    pay_pad = np.zeros((ng * GRP, 128, PAYW), np.uint8)
    pay_pad[:NB] = payload
    payload_gm = np.ascontiguousarray(
        pay_pad.reshape(ng, GRP, 128, PAYW).transpose(0, 2, 1, 3)
    ).reshape(ng, 128, GRP * PAYW).view(NP_FP8)
    return {"payload": payload_gm, "nodes_g": nodes_g_core}


def kernel(edge_data, node_data, W1, b1, W2, b2, receiver_ids, _trace=False):
    edge_data = np.asarray(edge_data, np.float32)
    node_data = np.asarray(node_data, np.float32)
    W1 = np.asarray(W1, np.float32)
    b1 = np.asarray(b1, np.float32)
    W2 = np.asarray(W2, np.float32)
    b2 = np.asarray(b2, np.float32)
    rid = np.asarray(receiver_ids).astype(np.int64)

    w1e_dev = np.ascontiguousarray(W1[:128]).astype(NP_BF16)
    w1n_dev = np.ascontiguousarray(W1[128:]).astype(NP_FP8)
    w2_dev = np.ascontiguousarray(
        W2.reshape(2, 128, Do).transpose(1, 0, 2).reshape(128, 2 * Do)).astype(NP_BF16)
    b1_dev = np.ascontiguousarray(b1.reshape(2, 128).T)
    b2_dev = np.ascontiguousarray(b2.reshape(128, 1))
    iota_dev = np.ascontiguousarray(np.broadcast_to(
        np.arange(1, SW + 1, dtype=np.uint8), (128, SW))).view(NP_FP8)

    ng = len(GROUPS)
    in_maps = []
    for core in range(NCORES):
        g, part = divmod(core, CPG)
        base = part * NPC
        sel = (rid[g] >= base) & (rid[g] < base + NPC)
        local = rid[g][sel] - base
        efeat = edge_data[g][sel]

        nd = np.zeros((ng * GRP * 128, Dv), np.float32)
        nd[:NPC] = node_data[g, base:base + NPC]
        # [ng, 128 d, GRP*128 n]: group-contiguous, feature-major
        nodes_g_core = np.ascontiguousarray(
            nd.reshape(ng, GRP * 128, Dv).transpose(0, 2, 1)).astype(NP_FP8)

        m = _prep_core(efeat, local, nodes_g_core)
        m.update({"w1e": w1e_dev, "w1n": w1n_dev, "w2": w2_dev, "b1": b1_dev,
                  "b2": b2_dev, "iota8": iota_dev})
        in_maps.append(m)

    nc = _build_nc()
    res = run_bass_kernel_spmd(nc, in_maps, core_ids=list(range(NCORES)),
                               trace=_trace)

    out = np.empty((B, N, Do), np.float32)
    for core in range(NCORES):
        g, part = divmod(core, CPG)
        og = res.results[core]["out_g"].astype(np.float32)  # [ng, 128 o, GRP*128 j]
        on = og.transpose(0, 2, 1).reshape(ng * GRP * 128, Do)
        out[g, part * NPC:(part + 1) * NPC] = on[:NPC]
    if _trace:
        kernel._last = res
    return out


# revision 18
# speedup vs baseline: 1.1385x; 1.1385x over previous
"""GNN NodeBlock (segment_sum scatter + 2-layer MLP) on 8 Trainium2 cores.

v2 strategy (edge/vertex partitioning by receiver range, fp8 payload):
 - 2 graphs x 4 cores each; core owns a 12500-node range and all edges
   whose receiver falls in that range.
 - Nodes grouped in 128-node blocks (98/core), each split into 4 stripes
   of 32 nodes. Host buckets edges by stripe; each stripe owns 3 fixed
   128-edge chunks (12 chunks/block). Stripes overflowing 384 edges are
   pre-compressed host-side (tail summed by receiver, <=32 rows).
 - Edge features are quantized to fp8 e4m3 with host-side error
   feedback: the fp8 rounding residual of each edge is carried into the
   next edge targeting the same (node, feature), so the device's exact
   fp32 PSUM sum of the quantized edges matches the fp32 sum to ~1 ulp
   of a single fp8 value. Halves DMA bytes vs bf16 at ~6e-3 rel error.
 - Edge ids are stored as fp8 BIT PATTERNS 1..32 (distinct values, not
   integers) and compared against a host-provided fp8 pattern iota via
   DVE is_equal to build the per-chunk one-hot [128 edge, 32 node].
 - Device, per chunk: scatter via matmul into agg_T [De, 128]
   (feature-major, fp8 lhsT/rhs, fp32 PSUM), then the MLP feature-major
   in bf16, batched over groups of 4 blocks.
 - Engine split: one-hot + agg PSUM->SBUF copy on Vector, relu+bias and
   out bias on Scalar, payload DMA on Sync queue, node/out DMA on
   GpSimd queue.
"""
import numpy as np
import ml_dtypes

import concourse.bacc as bacc
import concourse.mybir as mybir
from concourse.tile import TileContext
from concourse.bass_utils import run_bass_kernel_spmd

B, N, E = 2, 50000, 512000
De, Dv, H, Do = 128, 128, 256, 128
NCORES = 8
CPG = 4                    # cores per graph
NPC = N // CPG             # 12500 nodes per core
NB = (NPC + 127) // 128    # 98 blocks per core
SW = 32                    # stripe width: nodes per one-hot stripe
NSPB = 4                   # stripes per block
NSC = 11                   # 128-edge chunks per block: stripe blk%4 gets 2,
                           # the other three stripes get 3 (rotating short
                           # stripe; its tail is folded host-side)
IDOFF = NSC * 128          # ids start at column 1408
PAYW = IDOFF + NSC + 5     # 1424 bytes per partition row (5 pad)
GRP = 4                    # blocks per MLP group
GROUPS = [GRP] * (NB // GRP) + ([NB % GRP] if NB % GRP else [])

F32 = mybir.dt.float32
BF16 = mybir.dt.bfloat16
FP8 = mybir.dt.float8e4
NP_FP8 = ml_dtypes.float8_e4m3fn
NP_BF16 = ml_dtypes.bfloat16


def _build_nc():
    nc = bacc.Bacc("TRN2", target_bir_lowering=False)
    payload = nc.dram_tensor("payload", [len(GROUPS), 128, GRP * PAYW], FP8, kind="ExternalInput")
    nodes_g = nc.dram_tensor("nodes_g", [len(GROUPS), 128, GRP * 128], FP8, kind="ExternalInput")
    w1e = nc.dram_tensor("w1e", [128, 256], BF16, kind="ExternalInput")  # W1[:128] (agg half)
    w1n = nc.dram_tensor("w1n", [128, 256], FP8, kind="ExternalInput")   # W1[128:] (node half)
    w2 = nc.dram_tensor("w2", [128, 256], BF16, kind="ExternalInput")   # [p, hm*128+j] = W2[hm*128+p, j]
    b1 = nc.dram_tensor("b1", [128, 2], F32, kind="ExternalInput")      # [p, hm] = b1[hm*128+p]
    b2 = nc.dram_tensor("b2", [128, 1], F32, kind="ExternalInput")
    iota8 = nc.dram_tensor("iota8", [128, SW], FP8, kind="ExternalInput")
    out_g = nc.dram_tensor("out_g", [len(GROUPS), 128, GRP * 128], BF16, kind="ExternalOutput")

    with TileContext(nc) as tc:
        with tc.tile_pool(name="const", bufs=1) as cp, \
             tc.tile_pool(name="pay", bufs=12) as payp, \
             tc.tile_pool(name="oh", bufs=4) as ohp, \
             tc.tile_pool(name="nod", bufs=8) as nodp, \
             tc.tile_pool(name="agg4", bufs=3) as aggp, \
             tc.tile_pool(name="hsb", bufs=3) as hp, \
             tc.tile_pool(name="osb", bufs=4) as op_, \
             tc.tile_pool(name="psA", bufs=2, space="PSUM") as psA, \
             tc.tile_pool(name="psH", bufs=2, space="PSUM") as psH, \
             tc.tile_pool(name="psO", bufs=2, space="PSUM") as psO:
            # iota first (first one-hot depends on it), then weights on the
            # gpsimd queue so the sync queue starts streaming payload groups
            # immediately.
            iota_sb = cp.tile([128, SW], FP8)
            nc.sync.dma_start(out=iota_sb[:], in_=iota8[:, :])
            w1e_sb = cp.tile([128, 256], BF16)
            nc.gpsimd.dma_start(out=w1e_sb[:], in_=w1e[:, :])
            w1n_sb = cp.tile([128, 256], FP8)
            nc.gpsimd.dma_start(out=w1n_sb[:], in_=w1n[:, :])
            w2_sb = cp.tile([128, 256], BF16)
            nc.gpsimd.dma_start(out=w2_sb[:], in_=w2[:, :])
            b1_sb = cp.tile([128, 2], F32)
            nc.gpsimd.dma_start(out=b1_sb[:], in_=b1[:, :])
            b2_sb = cp.tile([128, 1], F32)
            nc.gpsimd.dma_start(out=b2_sb[:], in_=b2[:, :])

            # 3-stage software pipeline over groups. Per emission step t:
            #   load(t): DMAs + one-hot     (Vector does nothing else before
            #                                the next TT, so it runs at DMA
            #                                pace, never behind compute)
            #   scatter(t): 48 fp8 matmuls  (PE)
            #   mlp_h(t-1): CAST + h-MMs + relu  (deps finished during
            #                                     scatter(t-1)/this step)
            #   mlp_o(t-2): out-MMs
            #   fin(t-3): out bias + DMA
            # This keeps every engine FIFO free of waits on work emitted
            # later in its own stream, and the PE queue free of matmuls
            # whose inputs aren't already in flight on another engine.
            st = {}

            def load(gi, g_sz):
                s = st[gi] = {}
                nod = s["nod"] = nodp.tile([128, GRP * 128], FP8, name="nod")
                nc.gpsimd.dma_start(out=nod[:, :g_sz * 128],
                                    in_=nodes_g[gi, :, :g_sz * 128])
                pay_g = s["pay"] = payp.tile([128, GRP * PAYW], FP8, name="pay_g")
                nc.sync.dma_start(out=pay_g[:, :g_sz * PAYW],
                                  in_=payload[gi, :, :g_sz * PAYW])
                # one-hot for the whole group in one DVE op:
                # oh_g[p, g, c, n] = (ids[p, g, c] == iota[p, n])
                oh_g = s["oh"] = ohp.tile([128, GRP * NSC * SW], FP8, name="oh_g")
                ids_ap = (pay_g[:, :g_sz * PAYW]
                          .rearrange("p (g w) -> p g w", g=g_sz)
                          [:, :, IDOFF:IDOFF + NSC]
                          .to_broadcast([128, g_sz, NSC, SW]))
                iota_ap = (iota_sb[:, None, :SW]
                           .to_broadcast([128, g_sz * NSC, SW])
                           .rearrange("p (g c) n -> p g c n", g=g_sz))
                nc.vector.tensor_tensor(
                    out=oh_g[:, :g_sz * NSC * SW].rearrange(
                        "p (g c n) -> p g c n", g=g_sz, c=NSC),
                    in0=ids_ap,
                    in1=iota_ap,
                    op=mybir.AluOpType.is_equal,
                )

            def scatter(gi, g_sz):
                s = st[gi]
                pay_g, oh_g = s["pay"], s["oh"]
                agg_ps = s["agg_ps"] = psA.tile([128, GRP * 128], F32, space="PSUM", name="agg_ps")
                for g in range(g_sz):
                    po = g * PAYW
                    oo = g * NSC * SW
                    sh = (gi * GRP + g) % NSPB   # short stripe of this block
                    pc = 0
                    for sp in range(NSPB):
                        col = g * 128 + sp * SW
                        ncs = 2 if sp == sh else 3
                        for k in range(ncs):
                            nc.tensor.matmul(
                                out=agg_ps[:, col:col + SW],
                                lhsT=pay_g[:, po + pc * 128:po + (pc + 1) * 128],
                                rhs=oh_g[:, oo + pc * SW:oo + (pc + 1) * SW],
                                start=(k == 0),
                                stop=(k == ncs - 1),
                            )
                            pc += 1

            def mlp_h(gi, g_sz):
                s = st[gi]
                agg_sb = aggp.tile([128, GRP * 128], BF16)
                nc.scalar.copy(agg_sb[:, :g_sz * 128], s["agg_ps"][:, :g_sz * 128])
                hps = psH.tile([128, 2 * GRP * 128], F32, space="PSUM")
                for hm in range(2):
                    nc.tensor.matmul(
                        out=hps[:, hm * GRP * 128:hm * GRP * 128 + g_sz * 128],
                        lhsT=w1e_sb[:, hm * 128:(hm + 1) * 128],
                        rhs=agg_sb[:, :g_sz * 128],
                        start=True, stop=False,
                    )
                    nc.tensor.matmul(
                        out=hps[:, hm * GRP * 128:hm * GRP * 128 + g_sz * 128],
                        lhsT=w1n_sb[:, hm * 128:(hm + 1) * 128],
                        rhs=s["nod"][:, :g_sz * 128],
                        start=False, stop=True,
                    )
                h_sb = s["h"] = hp.tile([128, 2 * GRP * 128], BF16, name="h_sb")
                for hm in range(2):
                    nc.scalar.activation(
                        out=h_sb[:, hm * GRP * 128:hm * GRP * 128 + g_sz * 128],
                        in_=hps[:, hm * GRP * 128:hm * GRP * 128 + g_sz * 128],
                        func=mybir.ActivationFunctionType.Relu,
                        bias=b1_sb[:, hm:hm + 1],
                    )

            def mlp_o(gi, g_sz):
                s = st[gi]
                ops = s["ops"] = psO.tile([128, GRP * 128], F32, space="PSUM", name="ops")
                for hm in range(2):
                    nc.tensor.matmul(
                        out=ops[:, :g_sz * 128],
                        lhsT=w2_sb[:, hm * 128:(hm + 1) * 128],
                        rhs=s["h"][:, hm * GRP * 128:hm * GRP * 128 + g_sz * 128],
                        start=(hm == 0),
                        stop=(hm == 1),
                    )

            def fin(gi, g_sz):
                s = st[gi]
                o_sb = op_.tile([128, GRP * 128], BF16)
                if gi % 2 == 0:
                    nc.scalar.activation(
                        out=o_sb[:, :g_sz * 128],
                        in_=s["ops"][:, :g_sz * 128],
                        func=mybir.ActivationFunctionType.Identity,
                        bias=b2_sb[:, 0:1],
                    )
                else:
                    nc.vector.tensor_scalar(
                        out=o_sb[:, :g_sz * 128],
                        in0=s["ops"][:, :g_sz * 128],
                        scalar1=b2_sb[:, 0:1],
                        scalar2=None,
                        op0=mybir.AluOpType.add,
                    )
                nc.gpsimd.dma_start(out=out_g[gi, :, :g_sz * 128],
                                    in_=o_sb[:, :g_sz * 128])
                del st[gi]

            n = len(GROUPS)
            for t in range(n + 3):
                if t < n:
                    load(t, GROUPS[t])
                if t - 3 >= 0:
                    fin(t - 3, GROUPS[t - 3])
                if t < n:
                    scatter(t, GROUPS[t])
                if 0 <= t - 1 < n:
                    mlp_h(t - 1, GROUPS[t - 1])
                if 0 <= t - 2 < n:
                    mlp_o(t - 2, GROUPS[t - 2])
    nc.compile()
    return nc


def _quantize_feedback(efeat, local):
    """fp8-quantize edge rows with per-(node,feature) error feedback.

    Rows sharing a receiver node are quantized sequentially, carrying the
    rounding residual into the next row, so the fp32 sum of the quantized
    rows tracks the fp32 sum of the originals to ~one fp8 rounding error.
    """
    order = np.argsort(local, kind="stable")
    ls = local[order]
    counts = np.bincount(ls, minlength=NPC)
    offs = np.zeros(NPC, np.int64)
    np.cumsum(counts[:-1], out=offs[1:])
    rank = np.arange(len(ls)) - offs[ls]
    q = np.empty((len(ls), De), NP_FP8)
    carry = np.zeros((NPC, De), np.float32)
    es = efeat[order]
    maxr = int(rank.max()) + 1 if len(rank) else 0
    for r in range(maxr):
        m = rank == r
        idx = ls[m]
        v = es[m] + carry[idx]
        qq = v.astype(NP_FP8)
        carry[idx] = v - qq.astype(np.float32)
        q[m] = qq
    out = np.empty_like(q)
    out[order] = q
    return out


def _prep_core(efeat, local, nodes_g_core):
    """Build one core's payload from its edges (efeat fp32, local in [0,NPC))."""
    blk = local >> 7
    w128 = local & 127
    stripe = blk * NSPB + (w128 >> 5)   # global stripe id, [0, NB*4)
    w32 = w128 & 31
    nstripes = NB * NSPB
    sidx = np.arange(nstripes)
    s_of = sidx % NSPB
    b_of = sidx // NSPB
    nch_s = np.where((b_of % NSPB) == s_of, 2, 3)   # chunks per stripe
    cap = nch_s * 128
    keepcap = cap - SW

    counts = np.bincount(stripe, minlength=nstripes)
    order = np.argsort(stripe, kind="stable")
    str_s = stripe[order]
    w32_s = w32[order]
    ef_s = efeat[order]
    offs = np.zeros(nstripes, np.int64)
    np.cumsum(counts[:-1], out=offs[1:])
    pos = np.arange(len(str_s)) - offs[str_s]

    # fold the tail of any stripe exceeding its capacity: the edges beyond
    # keepcap are summed by receiver (<=32 rows) so the stripe fits
    foldm = (counts > cap)[str_s] & (pos >= keepcap[str_s])
    if foldm.any():
        key = str_s[foldm] * SW + w32_s[foldm]
        seg = np.zeros((nstripes * SW, De), np.float32)
        np.add.at(seg, key, ef_s[foldm])
        pres = np.zeros(nstripes * SW, bool)
        pres[key] = True
        nk = np.nonzero(pres)[0]
        # folded rows go FIRST so their (large) fp8 rounding residual is
        # absorbed by the node's remaining normal edges via error feedback
        ef_s = np.concatenate([seg[nk], ef_s[~foldm]])
        str_s = np.concatenate([(nk // SW).astype(str_s.dtype), str_s[~foldm]])
        w32_s = np.concatenate([(nk % SW).astype(w32_s.dtype), w32_s[~foldm]])
        order2 = np.argsort(str_s, kind="stable")
        str_s = str_s[order2]
        w32_s = w32_s[order2]
        ef_s = ef_s[order2]
        counts = np.bincount(str_s, minlength=nstripes)
        offs = np.zeros(nstripes, np.int64)
        np.cumsum(counts[:-1], out=offs[1:])
        pos = np.arange(len(str_s)) - offs[str_s]

    local_n = (str_s // NSPB) * 128 + (str_s % NSPB) * SW + w32_s
    qfeat = _quantize_feedback(ef_s, local_n)

    chunk_base = (np.cumsum(nch_s) - nch_s) - NSC * b_of  # chunk base within block
    pc = chunk_base[str_s] + pos // 128      # physical chunk 0..10
    prow = pos % 128
    blk_s = str_s // NSPB

    payload = np.zeros((NB, 128, PAYW), np.uint8)
    feat_view = payload[:, :, :IDOFF].reshape(NB, 128, NSC, 128).view(NP_FP8)
    feat_view[blk_s, prow, pc, :] = qfeat
    id_view = payload[:, :, IDOFF:IDOFF + NSC]
    id_view[blk_s, prow, pc] = (w32_s + 1).astype(np.uint8)  # fp8 patterns
    ng = len(GROUPS)
    pay_pad = np.zeros((ng * GRP, 128, PAYW), np.uint8)
    pay_pad[:NB] = payload
    payload_gm = np.ascontiguousarray(
        pay_pad.reshape(ng, GRP, 128, PAYW).transpose(0, 2, 1, 3)
    ).reshape(ng, 128, GRP * PAYW).view(NP_FP8)
    return {"payload": payload_gm, "nodes_g": nodes_g_core}


def kernel(edge_data, node_data, W1, b1, W2, b2, receiver_ids, _trace=False):
    edge_data = np.asarray(edge_data, np.float32)
    node_data = np.asarray(node_data, np.float32)
    W1 = np.asarray(W1, np.float32)
    b1 = np.asarray(b1, np.float32)
    W2 = np.asarray(W2, np.float32)
    b2 = np.asarray(b2, np.float32)
    rid = np.asarray(receiver_ids).astype(np.int64)

    w1e_dev = np.ascontiguousarray(W1[:128]).astype(NP_BF16)
    w1n_dev = np.ascontiguousarray(W1[128:]).astype(NP_FP8)
    w2_dev = np.ascontiguousarray(
        W2.reshape(2, 128, Do).transpose(1, 0, 2).reshape(128, 2 * Do)).astype(NP_BF16)
    b1_dev = np.ascontiguousarray(b1.reshape(2, 128).T)
    b2_dev = np.ascontiguousarray(b2.reshape(128, 1))
    iota_dev = np.ascontiguousarray(np.broadcast_to(
        np.arange(1, SW + 1, dtype=np.uint8), (128, SW))).view(NP_FP8)

    ng = len(GROUPS)
    in_maps = []
    for core in range(NCORES):
        g, part = divmod(core, CPG)
        base = part * NPC
        sel = (rid[g] >= base) & (rid[g] < base + NPC)
        local = rid[g][sel] - base
        efeat = edge_data[g][sel]

        nd = np.zeros((ng * GRP * 128, Dv), np.float32)
        nd[:NPC] = node_data[g, base:base + NPC]
        # [ng, 128 d, GRP*128 n]: group-contiguous, feature-major
        nodes_g_core = np.ascontiguousarray(
            nd.reshape(ng, GRP * 128, Dv).transpose(0, 2, 1)).astype(NP_FP8)

        m = _prep_core(efeat, local, nodes_g_core)
        m.update({"w1e": w1e_dev, "w1n": w1n_dev, "w2": w2_dev, "b1": b1_dev,
                  "b2": b2_dev, "iota8": iota_dev})
        in_maps.append(m)

    nc = _build_nc()
    res = run_bass_kernel_spmd(nc, in_maps, core_ids=list(range(NCORES)),
                               trace=_trace)

    out = np.empty((B, N, Do), np.float32)
    for core in range(NCORES):
        g, part = divmod(core, CPG)
        og = res.results[core]["out_g"].astype(np.float32)  # [ng, 128 o, GRP*128 j]
        on = og.transpose(0, 2, 1).reshape(ng * GRP * 128, Do)
        out[g, part * NPC:(part + 1) * NPC] = on[:NPC]
    if _trace:
        kernel._last = res
    return out
